# revision 33
# baseline (speedup 1.0000x reference)
"""Deep & Cross Network kernel for 8x Trainium2 NeuronCores (Bass/Tile).

Sharding: pure data-parallel over batch (512 rows/core); weights replicated
per core; no collectives (cost model charges >=15us constant per collective).

Host-side prep inside kernel() (layout/dtype only):
  - embedding lookup x0 = emb[ids] (pure gather), transpose per core to
    x0T [128, 13, 512] and quantize to fp8e4m3 at scale S=32
  - weights pre-tiled to SBUF-native lhsT layouts, fp8 at scale S
  - cross-net bias constants c10/c2s/cC collapse to three scalars (host dot
    products of weight-only data); they are 0 for the reference's zero
    cross_b and fold into immediates

Device math (per core, n=512, D=1664, S=32):
  All matmuls fp8 DoubleRow (2 k-tiles per instruction, 0.5 cyc/row).
  Deep tower weights-stationary: psum = (S x0)(S w) = S^2 * pre;
  evict h = relu(psum)/S stored fp8 (scale S) feeds the next layer.
  Cross net in batch-on-partition layout [128b, 4bt]:
    A = (x0q @ [w0 w1 w2 ow[:D]]) / S^2  -> [128, 4] per batch-tile
    t1 = 1+a0; t2 = t1*(1+a1)+c10; t3 = t2*(1+a2)+c2s
    logit = t3*a3 + cC+out_b + (h3 @ ow[D:]) ; out = sigmoid(logit)
  PE warmup matmuls during the DMA head burn the p-state ramp.
"""

import os
import sys
import numpy as np

for _p in ("/opt/trn_rl_repo",):
    if _p not in sys.path:
        sys.path.insert(0, _p)

import concourse.bass as bass
import concourse.tile as tile
from concourse import bacc, mybir
from concourse import bass_utils

F32 = mybir.dt.float32
FP8 = mybir.dt.float8e4
AF = mybir.ActivationFunctionType
ALU = mybir.AluOpType
DR = mybir.MatmulPerfMode.DoubleRow

B, F, E, H = 4096, 26, 64, 1000000
D = F * E            # 1664
NC = 8
BC = B // NC         # 512 rows per core
KD = D // 128        # 13 k-tiles over D
H1, H2, H3 = 1024, 512, 256
M1, M2, M3 = H1 // 128, H2 // 128, H3 // 128
K2P, K3P = H1 // 256, H2 // 256   # DR k-pairs for L2/L3
NP1 = (KD - 1) // 2               # 6 DR pairs for L1 (kt0 single first)
S = 32.0
INV_S = 1.0 / S
INV_S2 = 1.0 / (S * S)
NWARM = int(os.environ.get("K_WARM", "22"))

_CACHE = {}


def _emit(tc, flags):
    nc = tc.nc
    a = _CACHE["aps"]
    c10, c2s, cCb, has_bias = flags
    R = int(os.environ.get("K_REPEAT", "1"))
    BODY = os.environ.get("K_BODY", "full")  # full | compute | dma

    with (
        tc.tile_pool(name="const", bufs=1) as cpool,
        tc.tile_pool(name="act", bufs=1) as apool,
        tc.tile_pool(name="psmm", bufs=7, space="PSUM") as psmm,
        tc.tile_pool(name="pssm", bufs=1, space="PSUM") as pssm,
    ):
        def _loads():
            # ws: w4 in [:, 0:13, :], ow2 in [:, 13:15, 0]
            ws_sb = cpool.tile([128, KD + 2, 4], FP8, tag="ws")
            nc.sync.dma_start(ws_sb[:], a["wsx"][:])
            x0_sb = cpool.tile([128, KD, BC], FP8, tag="x0")
            w1_sb = cpool.tile([128, KD, M1, 128], FP8, tag="w1")
            # band 0 = kt0, bands j>=1 = kt pair (2j-1, 2j); loads chunked
            # by band groups, interleaved x0/w1 so bands stream in order
            for lo, hi in ((0, 1), (1, 3), (3, 5), (5, 9), (9, 11), (11, 13)):
                nc.sync.dma_start(x0_sb[:, lo:hi, :], a["x0T"][:, lo:hi, :])
                nc.sync.dma_start(w1_sb[:, lo:hi, :, :], a["w1x"][:, lo:hi, :, :])
            w2_sb = cpool.tile([128, M2, K2P, 2, 128], FP8, tag="w2")
            nc.sync.dma_start(w2_sb[:], a["w2x"][:])
            w3_sb = cpool.tile([128, M3, K3P, 2, 128], FP8, tag="w3")
            nc.sync.dma_start(w3_sb[:], a["w3x"][:])
            T = dict(ws_sb=ws_sb, x0_sb=x0_sb, w1_sb=w1_sb,
                     w2_sb=w2_sb, w3_sb=w3_sb)
            if has_bias:
                ball_sb = cpool.tile([128, M1 + M2 + M3], F32, tag="ball")
                nc.sync.dma_start(ball_sb[:], a["ballx"][:])
                T["ball_sb"] = ball_sb
            return T

        def _compute(T):
            ws_sb = T["ws_sb"]; x0_sb = T["x0_sb"]; w1_sb = T["w1_sb"]
            w2_sb = T["w2_sb"]; w3_sb = T["w3_sb"]
            ball = T.get("ball_sb")

            def bias_ap(layer_off, m):
                return ball[:, layer_off + m:layer_off + m + 1]

            # --- engine warmups (all independent of loads) ---
            warm = apool.tile([128, 128], FP8, tag="warm")
            nc.gpsimd.memset(warm[:], 0.0)
            zero_sb = apool.tile([128, 1], F32, tag="zero")
            nc.vector.memset(zero_sb[:], 0.0)
            # ACT function tables (Sigmoid set also contains Relu/Copy)
            wa = apool.tile([128, 1], F32, tag="wa")
            nc.scalar.activation(out=wa[:], in_=zero_sb[:], func=AF.Sigmoid,
                                 bias=zero_sb[:])
            wr = apool.tile([128, 1], F32, tag="wr")
            nc.scalar.activation(out=wr[:], in_=zero_sb[:], func=AF.Relu,
                                 bias=zero_sb[:])
            # single PSUM bank shared by warmup/matvec/head outputs. A
            # start=True zero-marks the whole bank here, so each region gets
            # exactly one group start per live window; the warmup sweep also
            # writes zeros over the full bank so start=False accumulation
            # into fresh regions is safe under element-wise-zero semantics.
            arena = pssm.tile([128, 512], F32, tag="sm", name="arena")
            # PE p-state ramp burn: back-to-back junk matmuls sweeping arena
            for i in range(max(NWARM, 4)):
                c = (i % 4) * 128
                nc.tensor.matmul(out=arena[:, c:c + 128], lhsT=warm[:],
                                 rhs=warm[:], start=True, stop=True,
                                 skip_group_check=True)

            # --- band helpers: band 0 = kt0 single, band j = DR pair ---
            def mm_band(j, ps, lhsT_kt, rhs_kt, stop, start=None, skip=False):
                """lhsT_kt/rhs_kt: callables band-slice -> AP"""
                if start is None:
                    start = j == 0
                if j == 0:
                    nc.tensor.matmul(out=ps, lhsT=lhsT_kt(0, 1),
                                     rhs=rhs_kt(0, 1), start=start, stop=stop,
                                     skip_group_check=skip)
                else:
                    lo, hi = 2 * j - 1, 2 * j + 1
                    nc.tensor.matmul(out=ps, lhsT=lhsT_kt(lo, hi),
                                     rhs=rhs_kt(lo, hi), start=start,
                                     stop=stop, perf_mode=DR,
                                     skip_group_check=skip)

            def x0sl(lo, hi):
                return x0_sb[:, lo:hi, :] if hi - lo == 2 else x0_sb[:, lo, :]

            # --- eviction helpers ---
            h1T = apool.tile([128, M1, BC], FP8, tag="h1T")
            h2T = apool.tile([128, M2, BC], FP8, tag="h2T")
            h3T = apool.tile([128, M3, BC], FP8, tag="h3T")

            def evict(dst, ps, eng, loff, m):
                """dst = relu(ps * 1/S [+ S*b]) on the given engine."""
                if eng == "act":
                    nc.scalar.activation(
                        out=dst, in_=ps, func=AF.Relu, scale=INV_S,
                        bias=bias_ap(loff, m) if has_bias else zero_sb[:])
                elif not has_bias:
                    nc.vector.tensor_scalar(
                        out=dst, in0=ps, scalar1=INV_S,
                        scalar2=0.0, op0=ALU.mult, op1=ALU.max)
                else:
                    nc.vector.tensor_scalar(
                        out=dst, in0=ps, scalar1=INV_S,
                        scalar2=bias_ap(loff, m), op0=ALU.mult, op1=ALU.add)
                    nc.vector.tensor_scalar_max(out=dst, in0=dst, scalar1=0.0)

            def evict_split(dst3, ps, loff, m):
                """latency-critical eviction: halves on ACT + DVE."""
                h = BC // 2
                evict(dst3[:, m, 0:h], ps[:, 0:h], "act", loff, m)
                evict(dst3[:, m, h:BC], ps[:, h:BC], "dve", loff, m)

            # --- L1: 8 PSUM banks (7 pool + arena), band-major bands 0-5,
            # band 6 m-major with immediate staggered evictions ---
            l1ps = [psmm.tile([128, BC], F32, tag="mm", name=f"l1_{m}")
                    for m in range(7)] + [arena]

            def w1lh(lo, hi, m):
                return (w1_sb[:, lo:hi, m, :] if hi - lo == 2
                        else w1_sb[:, lo, m, :])

            for j in range(NP1 - 2):
                for m in range(8):
                    mm_band(j, l1ps[m][:], lambda lo, hi, m=m: w1lh(lo, hi, m),
                            x0sl, stop=False)
            # last three bands m-major so evictions stagger into the L1 tail
            for m in range(8):
                for j in range(NP1 - 2, NP1 + 1):
                    mm_band(j, l1ps[m][:],
                            lambda lo, hi, m=m: w1lh(lo, hi, m), x0sl,
                            stop=(j == NP1))
                evict(h1T[:, m, :], l1ps[m][:], ("act", "dve")[m % 2], 0, m)

            # --- cross matvec (batch-on-partition), SBUF-fed post-L1;
            # accumulates in a recycled bank (one bank-wide group) ---
            atps = psmm.tile([128, BC], F32, tag="mm", name="atps")
            at_ps = atps[:, 0:16]
            for j in range(NP1 + 1):
                for bt in range(4):
                    bsl = slice(bt * 128, bt * 128 + 128)

                    def xlh(lo, hi, bsl=bsl):
                        return (x0_sb[:, lo:hi, bsl] if hi - lo == 2
                                else x0_sb[:, lo, bsl])

                    def wrh(lo, hi):
                        return (ws_sb[:, lo:hi, :] if hi - lo == 2
                                else ws_sb[:, lo, :])

                    mm_band(j, at_ps[:, 4 * bt:4 * bt + 4], xlh, wrh,
                            stop=(j == NP1 and bt == 3),
                            start=(j == 0 and bt == 0), skip=True)

            # --- L2 in two k-phases; phase 2 staggers h2 evictions and
            # interleaves L3 pairs as their h2 inputs land ---
            l2ps = [psmm.tile([128, BC], F32, tag="mm", name=f"l2_{m}")
                    for m in range(M2)]
            for m in range(M2):
                for t in range(2):
                    nc.tensor.matmul(
                        out=l2ps[m][:], lhsT=w2_sb[:, m, t, :, :],
                        rhs=h1T[:, 2 * t:2 * t + 2, :],
                        start=(t == 0), stop=False, perf_mode=DR)
            l3ps = [psmm.tile([128, BC], F32, tag="mm", name=f"l3_{m}")
                    for m in range(M3)]

            def l2_ph2(m):
                for t in range(2, K2P):
                    nc.tensor.matmul(
                        out=l2ps[m][:], lhsT=w2_sb[:, m, t, :, :],
                        rhs=h1T[:, 2 * t:2 * t + 2, :],
                        start=False, stop=(t == K2P - 1), perf_mode=DR)
                evict_split(h2T, l2ps[m][:], M1, m)

            def l3_pair(t, stop):
                for m in range(M3):
                    nc.tensor.matmul(
                        out=l3ps[m][:], lhsT=w3_sb[:, m, t, :, :],
                        rhs=h2T[:, 2 * t:2 * t + 2, :],
                        start=(t == 0), stop=stop, perf_mode=DR)

            l2_ph2(0)
            l2_ph2(1)
            # at eviction (DVE, reads PSUM) + cross recurrence. Fast path
            # runs the recurrence on the otherwise-idle Pool engine with
            # tensor_tensor ops only (Pool ISA has no tensor_scalar);
            # general path (nonzero cross_b consts) falls back to DVE.
            # at_sb layout [128, j, bt] so A_j slices are contiguous
            # (GPSIMD chokes on strided reads); DVE does the strided write
            at_sb = apool.tile([128, 4, 4], F32, tag="at")
            for bt in range(4):
                nc.vector.tensor_scalar_mul(
                    at_sb[:, :, bt], at_ps[:, 4 * bt:4 * bt + 4], INV_S2)
            A = [at_sb[:, jj, :] for jj in range(4)]
            v = apool.tile([128, 4], F32, tag="v")
            if c10 == 0.0 and c2s == 0.0:
                ones = apool.tile([128, 4], F32, tag="ones")
                nc.gpsimd.memset(ones[:], 1.0)
                t1 = apool.tile([128, 4], F32, tag="t1")
                nc.gpsimd.tensor_tensor(out=t1[:], in0=A[0], in1=ones[:],
                                        op=ALU.add)
                u1 = apool.tile([128, 4], F32, tag="u1")
                nc.gpsimd.tensor_tensor(out=u1[:], in0=A[1], in1=ones[:],
                                        op=ALU.add)
                t2 = apool.tile([128, 4], F32, tag="t2")
                nc.gpsimd.tensor_tensor(out=t2[:], in0=t1[:], in1=u1[:],
                                        op=ALU.mult)
                u2 = apool.tile([128, 4], F32, tag="u2")
                nc.gpsimd.tensor_tensor(out=u2[:], in0=A[2], in1=ones[:],
                                        op=ALU.add)
                t3 = apool.tile([128, 4], F32, tag="t3")
                nc.gpsimd.tensor_tensor(out=t3[:], in0=t2[:], in1=u2[:],
                                        op=ALU.mult)
                nc.gpsimd.tensor_tensor(out=v[:], in0=t3[:], in1=A[3],
                                        op=ALU.mult)
            else:
                t1 = apool.tile([128, 4], F32, tag="t1")
                nc.vector.tensor_scalar_add(t1[:], A[0], 1.0)
                t2 = apool.tile([128, 4], F32, tag="t2")
                nc.vector.scalar_tensor_tensor(
                    out=t2[:], in0=A[1], scalar=1.0, in1=t1[:],
                    op0=ALU.add, op1=ALU.mult)
                if c10 != 0.0:
                    nc.vector.tensor_scalar_add(t2[:], t2[:], c10)
                t3 = apool.tile([128, 4], F32, tag="t3")
                nc.vector.scalar_tensor_tensor(
                    out=t3[:], in0=A[2], scalar=1.0, in1=t2[:],
                    op0=ALU.add, op1=ALU.mult)
                if c2s != 0.0:
                    nc.vector.tensor_scalar_add(t3[:], t3[:], c2s)
                nc.vector.tensor_tensor(out=v[:], in0=t3[:], in1=A[3],
                                        op=ALU.mult)


            l3_pair(0, stop=False)
            l2_ph2(2)
            l2_ph2(3)
            l3_pair(1, stop=True)
            for m in range(M3):
                evict_split(h3T, l3ps[m][:], M1 + M2, m)

            # --- head: hd[b] = ow2^T h3[b] per batch tile (DR, N=1);
            # hdps recycles the matvec bank: pool WAR on the at eviction
            # orders head's bank reuse after the at values are read out ---
            hdps = psmm.tile([128, BC], F32, tag="mm", name="hdps")
            hd_ps = hdps[:, 0:4]
            for bt in range(4):
                bsl = slice(bt * 128, bt * 128 + 128)
                nc.tensor.matmul(
                    out=hd_ps[:, bt:bt + 1], lhsT=h3T[:, 0:2, bsl],
                    rhs=ws_sb[:, KD:KD + 2, 0:1], start=(bt == 0),
                    stop=(bt == 3), perf_mode=DR, skip_group_check=True)

            # --- final combine + sigmoid + out ---
            lg = apool.tile([128, 4], F32, tag="lg")
            nc.vector.scalar_tensor_tensor(
                out=lg[:], in0=hd_ps[:], scalar=INV_S2, in1=v[:],
                op0=ALU.mult, op1=ALU.add)
            if cCb != 0.0:
                nc.vector.tensor_scalar_add(lg[:], lg[:], cCb)
            res = apool.tile([128, 4], F32, tag="res")
            nc.scalar.activation(out=res[:], in_=lg[:], func=AF.Sigmoid,
                                 bias=zero_sb[:])
            nc.sync.dma_start(a["out"][:], res[:])

        if R == 1:
            _compute(_loads())
        elif BODY == "full":
            with tc.For_i(0, R, 1):
                _compute(_loads())
        elif BODY == "compute":
            T = _loads()
            with tc.For_i(0, R, 1):
                _compute(T)
        elif BODY == "dma":
            with tc.For_i(0, R, 1):
                _loads()
        else:
            raise ValueError(BODY)


def build_program(flags):
    key = ("nc", flags, os.environ.get("K_REPEAT", "1"),
           os.environ.get("K_BODY", "full"))
    if key in _CACHE:
        return _CACHE[key]
    nc = bacc.Bacc("TRN2", target_bir_lowering=False, debug=False,
                   num_devices=NC)
    aps = {}

    def din(name, shape, dt):
        aps[name] = nc.dram_tensor(name, shape, dt, kind="ExternalInput").ap()

    din("x0T", [128, KD, BC], FP8)
    din("w1x", [128, KD, M1, 128], FP8)
    din("w2x", [128, M2, K2P, 2, 128], FP8)
    din("w3x", [128, M3, K3P, 2, 128], FP8)
    din("wsx", [128, KD + 2, 4], FP8)
    if flags[3]:
        din("ballx", [128, M1 + M2 + M3], F32)
    aps["out"] = nc.dram_tensor("out", [1 * 128, 4], F32,
                                kind="ExternalOutput").ap()
    _CACHE["aps"] = aps

    with tile.TileContext(nc) as tc:
        _emit(tc, flags)
    nc.compile()
    _CACHE[key] = nc
    _CACHE["nc"] = nc  # most-recent program, for test harness introspection
    return nc


def _q(x):
    import ml_dtypes
    return (np.asarray(x, np.float32) * S).astype(ml_dtypes.float8_e4m3fn)


def prepare_in_maps(inputs):
    ids = np.asarray(inputs["ids"]).astype(np.int64)
    emb = np.asarray(inputs["emb"], dtype=np.float32)
    cross_w = np.asarray(inputs["cross_w"], dtype=np.float32)
    cross_b = np.asarray(inputs["cross_b"], dtype=np.float32)
    w1 = np.asarray(inputs["w1"], dtype=np.float32)
    w2 = np.asarray(inputs["w2"], dtype=np.float32)
    w3 = np.asarray(inputs["w3"], dtype=np.float32)
    b1 = np.asarray(inputs["b1"], dtype=np.float32)
    b2 = np.asarray(inputs["b2"], dtype=np.float32)
    b3 = np.asarray(inputs["b3"], dtype=np.float32)
    out_w = np.asarray(inputs["out_w"], dtype=np.float32)
    out_b = float(np.asarray(inputs["out_b"], dtype=np.float32))

    # cross-net constants (weight-only): CB[j,i] = W4[:,j] . cross_b[i]
    c10 = float(cross_w[1] @ cross_b[0])
    c2s = float(cross_w[2] @ (cross_b[0] + cross_b[1]))
    cCb = float(out_w[:D, 0] @ cross_b.sum(axis=0)) + out_b
    has_bias = bool(np.any(b1) or np.any(b2) or np.any(b3))
    flags = (c10, c2s, cCb, has_bias)

    x0 = emb[ids.reshape(-1)].reshape(B, D)  # [4096, 1664] f32

    w4 = np.concatenate([cross_w.T, out_w[:D].reshape(D, 1)], axis=1)
    # [K, M] -> [128, kt, M-free] k-major  (w1: [128, kt, m, 128])
    w1x = np.ascontiguousarray(
        _q(w1).reshape(KD, 128, M1, 128).transpose(1, 0, 2, 3))
    # [K, M] -> [128, m, kpair, 2, 128]
    w2x = np.ascontiguousarray(
        _q(w2).reshape(K2P, 2, 128, M2, 128).transpose(2, 3, 0, 1, 4))
    w3x = np.ascontiguousarray(
        _q(w3).reshape(K3P, 2, 128, M3, 128).transpose(2, 3, 0, 1, 4))
    import ml_dtypes
    wsx = np.zeros((128, KD + 2, 4), dtype=ml_dtypes.float8_e4m3fn)
    wsx[:, 0:KD, :] = _q(w4).reshape(KD, 128, 4).transpose(1, 0, 2)
    # ow2 as 2 k-tiles of 1 col: [p, kt] at ws[:, 13:15, 0]
    wsx[:, KD:KD + 2, 0] = _q(out_w[D:]).reshape(M3, 128).T
    shared = dict(w1x=w1x, w2x=w2x, w3x=w3x, wsx=np.ascontiguousarray(wsx))
    if has_bias:
        ballx = np.zeros((128, M1 + M2 + M3), dtype=np.float32)
        ballx[:, 0:M1] = S * b1.reshape(M1, 128).T
        ballx[:, M1:M1 + M2] = S * b2.reshape(M2, 128).T
        ballx[:, M1 + M2:] = S * b3.reshape(M3, 128).T
        shared["ballx"] = np.ascontiguousarray(ballx)

    in_maps = []
    for c in range(NC):
        xc = _q(x0[c * BC:(c + 1) * BC])                  # [512, 1664] fp8
        x0T = np.ascontiguousarray(
            xc.reshape(BC, KD, 128).transpose(2, 1, 0))   # [128, 13, 512]
        in_maps.append(dict(x0T=x0T, **shared))
    return in_maps, flags


def kernel(**inputs):
    in_maps, flags = prepare_in_maps(inputs)
    nc = build_program(flags)
    res = bass_utils.run_bass_kernel_spmd(nc, in_maps, core_ids=list(range(NC)))
    out = np.empty((NC, BC), dtype=np.float32)
    for c in range(NC):
        o = res.results[c]["out"]            # [128, 4] -> example bt*128+p
        out[c] = o.T.reshape(BC)
    return out.reshape(B, 1).astype(np.float32)


# revision 34
# speedup vs baseline: 1.0201x; 1.0201x over previous
"""Deep & Cross Network kernel for 8x Trainium2 NeuronCores (Bass/Tile).

Sharding: pure data-parallel over batch (512 rows/core); weights replicated
per core; no collectives (cost model charges >=15us constant per collective).

Host-side prep inside kernel() (layout/dtype only):
  - embedding lookup x0 = emb[ids] (pure gather), transpose per core to
    x0T [128, 13, 512] and quantize to fp8e4m3 at scale S=32
  - weights pre-tiled to SBUF-native lhsT layouts, fp8 at scale S
  - cross-net bias constants c10/c2s/cC collapse to three scalars (host dot
    products of weight-only data); they are 0 for the reference's zero
    cross_b and fold into immediates

Device math (per core, n=512, D=1664, S=32):
  All matmuls fp8 DoubleRow (2 k-tiles per instruction, 0.5 cyc/row).
  Deep tower weights-stationary: psum = (S x0)(S w) = S^2 * pre;
  evict h = relu(psum)/S stored fp8 (scale S) feeds the next layer.
  Cross net in batch-on-partition layout [128b, 4bt]:
    A = (x0q @ [w0 w1 w2 ow[:D]]) / S^2  -> [128, 4] per batch-tile
    t1 = 1+a0; t2 = t1*(1+a1)+c10; t3 = t2*(1+a2)+c2s
    logit = t3*a3 + cC+out_b + (h3 @ ow[D:]) ; out = sigmoid(logit)
  PE warmup matmuls during the DMA head burn the p-state ramp.
"""

import os
import sys
import numpy as np

for _p in ("/opt/trn_rl_repo",):
    if _p not in sys.path:
        sys.path.insert(0, _p)

import concourse.bass as bass
import concourse.tile as tile
from concourse import bacc, mybir
from concourse import bass_utils

F32 = mybir.dt.float32
FP8 = mybir.dt.float8e4
AF = mybir.ActivationFunctionType
ALU = mybir.AluOpType
DR = mybir.MatmulPerfMode.DoubleRow

B, F, E, H = 4096, 26, 64, 1000000
D = F * E            # 1664
NC = 8
BC = B // NC         # 512 rows per core
KD = D // 128        # 13 k-tiles over D
H1, H2, H3 = 1024, 512, 256
M1, M2, M3 = H1 // 128, H2 // 128, H3 // 128
K2P, K3P = H1 // 256, H2 // 256   # DR k-pairs for L2/L3
NP1 = (KD - 1) // 2               # 6 DR pairs for L1 (kt0 single first)
S = 32.0
INV_S = 1.0 / S
INV_S2 = 1.0 / (S * S)
NWARM = int(os.environ.get("K_WARM", "24"))

_CACHE = {}


def _emit(tc, flags):
    nc = tc.nc
    a = _CACHE["aps"]
    c10, c2s, cCb, has_bias = flags
    R = int(os.environ.get("K_REPEAT", "1"))
    BODY = os.environ.get("K_BODY", "full")  # full | compute | dma

    with (
        tc.tile_pool(name="const", bufs=1) as cpool,
        tc.tile_pool(name="act", bufs=1) as apool,
        tc.tile_pool(name="psmm", bufs=7, space="PSUM") as psmm,
        tc.tile_pool(name="pssm", bufs=1, space="PSUM") as pssm,
    ):
        def _loads():
            # ws: w4 in [:, 0:13, :], ow2 in [:, 13:15, 0]
            ws_sb = cpool.tile([128, KD + 2, 4], FP8, tag="ws")
            nc.sync.dma_start(ws_sb[:], a["wsx"][:])
            x0_sb = cpool.tile([128, KD, BC], FP8, tag="x0")
            w1_sb = cpool.tile([128, KD, M1, 128], FP8, tag="w1")
            # band 0 = kt0, bands j>=1 = kt pair (2j-1, 2j); loads chunked
            # by band groups, interleaved x0/w1 so bands stream in order
            for lo, hi in ((0, 1), (1, 5), (5, 9), (9, 13)):
                nc.sync.dma_start(x0_sb[:, lo:hi, :], a["x0T"][:, lo:hi, :])
                nc.sync.dma_start(w1_sb[:, lo:hi, :, :], a["w1x"][:, lo:hi, :, :])
            w2_sb = cpool.tile([128, M2, K2P, 2, 128], FP8, tag="w2")
            nc.sync.dma_start(w2_sb[:], a["w2x"][:])
            w3_sb = cpool.tile([128, M3, K3P, 2, 128], FP8, tag="w3")
            nc.sync.dma_start(w3_sb[:], a["w3x"][:])
            T = dict(ws_sb=ws_sb, x0_sb=x0_sb, w1_sb=w1_sb,
                     w2_sb=w2_sb, w3_sb=w3_sb)
            if has_bias:
                ball_sb = cpool.tile([128, M1 + M2 + M3], F32, tag="ball")
                nc.sync.dma_start(ball_sb[:], a["ballx"][:])
                T["ball_sb"] = ball_sb
            return T

        def _compute(T):
            ws_sb = T["ws_sb"]; x0_sb = T["x0_sb"]; w1_sb = T["w1_sb"]
            w2_sb = T["w2_sb"]; w3_sb = T["w3_sb"]
            ball = T.get("ball_sb")

            def bias_ap(layer_off, m):
                return ball[:, layer_off + m:layer_off + m + 1]

            # --- engine warmups (all independent of loads) ---
            warm = apool.tile([128, 128], FP8, tag="warm")
            nc.gpsimd.memset(warm[:], 0.0)
            zero_sb = apool.tile([128, 1], F32, tag="zero")
            nc.vector.memset(zero_sb[:], 0.0)
            # ACT function tables (Sigmoid set also contains Relu/Copy)
            wa = apool.tile([128, 1], F32, tag="wa")
            nc.scalar.activation(out=wa[:], in_=zero_sb[:], func=AF.Sigmoid,
                                 bias=zero_sb[:])
            wr = apool.tile([128, 1], F32, tag="wr")
            nc.scalar.activation(out=wr[:], in_=zero_sb[:], func=AF.Relu,
                                 bias=zero_sb[:])
            # single PSUM bank shared by warmup/matvec/head outputs. A
            # start=True zero-marks the whole bank here, so each region gets
            # exactly one group start per live window; the warmup sweep also
            # writes zeros over the full bank so start=False accumulation
            # into fresh regions is safe under element-wise-zero semantics.
            arena = pssm.tile([128, 512], F32, tag="sm", name="arena")
            # PE p-state ramp burn: back-to-back junk matmuls sweeping arena
            for i in range(max(NWARM, 4)):
                c = (i % 4) * 128
                nc.tensor.matmul(out=arena[:, c:c + 128], lhsT=warm[:],
                                 rhs=warm[:], start=True, stop=True,
                                 skip_group_check=True)

            # --- band helpers: band 0 = kt0 single, band j = DR pair ---
            def mm_band(j, ps, lhsT_kt, rhs_kt, stop, start=None, skip=False):
                """lhsT_kt/rhs_kt: callables band-slice -> AP"""
                if start is None:
                    start = j == 0
                if j == 0:
                    nc.tensor.matmul(out=ps, lhsT=lhsT_kt(0, 1),
                                     rhs=rhs_kt(0, 1), start=start, stop=stop,
                                     skip_group_check=skip)
                else:
                    lo, hi = 2 * j - 1, 2 * j + 1
                    nc.tensor.matmul(out=ps, lhsT=lhsT_kt(lo, hi),
                                     rhs=rhs_kt(lo, hi), start=start,
                                     stop=stop, perf_mode=DR,
                                     skip_group_check=skip)

            def x0sl(lo, hi):
                return x0_sb[:, lo:hi, :] if hi - lo == 2 else x0_sb[:, lo, :]

            # --- eviction helpers ---
            h1T = apool.tile([128, M1, BC], FP8, tag="h1T")
            h2T = apool.tile([128, M2, BC], FP8, tag="h2T")
            h3T = apool.tile([128, M3, BC], FP8, tag="h3T")

            def evict(dst, ps, eng, loff, m):
                """dst = relu(ps * 1/S [+ S*b]) on the given engine."""
                if eng == "act":
                    nc.scalar.activation(
                        out=dst, in_=ps, func=AF.Relu, scale=INV_S,
                        bias=bias_ap(loff, m) if has_bias else zero_sb[:])
                elif not has_bias:
                    nc.vector.tensor_scalar(
                        out=dst, in0=ps, scalar1=INV_S,
                        scalar2=0.0, op0=ALU.mult, op1=ALU.max)
                else:
                    nc.vector.tensor_scalar(
                        out=dst, in0=ps, scalar1=INV_S,
                        scalar2=bias_ap(loff, m), op0=ALU.mult, op1=ALU.add)
                    nc.vector.tensor_scalar_max(out=dst, in0=dst, scalar1=0.0)

            def evict_split(dst3, ps, loff, m):
                """latency-critical eviction: halves on ACT + DVE."""
                h = BC // 2
                evict(dst3[:, m, 0:h], ps[:, 0:h], "act", loff, m)
                evict(dst3[:, m, h:BC], ps[:, h:BC], "dve", loff, m)

            # --- L1: 8 PSUM banks (7 pool + arena), band-major bands 0-5,
            # band 6 m-major with immediate staggered evictions ---
            l1ps = [psmm.tile([128, BC], F32, tag="mm", name=f"l1_{m}")
                    for m in range(7)] + [arena]

            def w1lh(lo, hi, m):
                return (w1_sb[:, lo:hi, m, :] if hi - lo == 2
                        else w1_sb[:, lo, m, :])

            for j in range(NP1 - 2):
                for m in range(8):
                    mm_band(j, l1ps[m][:], lambda lo, hi, m=m: w1lh(lo, hi, m),
                            x0sl, stop=False)
            # last three bands m-major so evictions stagger into the L1 tail
            for m in range(8):
                for j in range(NP1 - 2, NP1 + 1):
                    mm_band(j, l1ps[m][:],
                            lambda lo, hi, m=m: w1lh(lo, hi, m), x0sl,
                            stop=(j == NP1))
                evict(h1T[:, m, :], l1ps[m][:], ("act", "dve")[m % 2], 0, m)

            # --- cross matvec (batch-on-partition), SBUF-fed post-L1;
            # accumulates in a recycled bank (one bank-wide group) ---
            atps = psmm.tile([128, BC], F32, tag="mm", name="atps")
            at_ps = atps[:, 0:16]
            for j in range(NP1 + 1):
                for bt in range(4):
                    bsl = slice(bt * 128, bt * 128 + 128)

                    def xlh(lo, hi, bsl=bsl):
                        return (x0_sb[:, lo:hi, bsl] if hi - lo == 2
                                else x0_sb[:, lo, bsl])

                    def wrh(lo, hi):
                        return (ws_sb[:, lo:hi, :] if hi - lo == 2
                                else ws_sb[:, lo, :])

                    mm_band(j, at_ps[:, 4 * bt:4 * bt + 4], xlh, wrh,
                            stop=(j == NP1 and bt == 3),
                            start=(j == 0 and bt == 0), skip=True)

            # --- L2 in two k-phases; phase 2 staggers h2 evictions and
            # interleaves L3 pairs as their h2 inputs land ---
            l2ps = [psmm.tile([128, BC], F32, tag="mm", name=f"l2_{m}")
                    for m in range(M2)]
            for m in range(M2):
                for t in range(2):
                    nc.tensor.matmul(
                        out=l2ps[m][:], lhsT=w2_sb[:, m, t, :, :],
                        rhs=h1T[:, 2 * t:2 * t + 2, :],
                        start=(t == 0), stop=False, perf_mode=DR)
            l3ps = [psmm.tile([128, BC], F32, tag="mm", name=f"l3_{m}")
                    for m in range(M3)]

            def l2_ph2(m):
                for t in range(2, K2P):
                    nc.tensor.matmul(
                        out=l2ps[m][:], lhsT=w2_sb[:, m, t, :, :],
                        rhs=h1T[:, 2 * t:2 * t + 2, :],
                        start=False, stop=(t == K2P - 1), perf_mode=DR)
                evict_split(h2T, l2ps[m][:], M1, m)

            def l3_pair(t, stop):
                for m in range(M3):
                    nc.tensor.matmul(
                        out=l3ps[m][:], lhsT=w3_sb[:, m, t, :, :],
                        rhs=h2T[:, 2 * t:2 * t + 2, :],
                        start=(t == 0), stop=stop, perf_mode=DR)

            l2_ph2(0)
            l2_ph2(1)
            # at eviction (DVE, reads PSUM) + cross recurrence. Fast path
            # runs the recurrence on the otherwise-idle Pool engine with
            # tensor_tensor ops only (Pool ISA has no tensor_scalar);
            # general path (nonzero cross_b consts) falls back to DVE.
            # at_sb layout [128, j, bt] so A_j slices are contiguous
            # (GPSIMD chokes on strided reads); DVE does the strided write
            at_sb = apool.tile([128, 4, 4], F32, tag="at")
            for bt in range(4):
                nc.vector.tensor_scalar_mul(
                    at_sb[:, :, bt], at_ps[:, 4 * bt:4 * bt + 4], INV_S2)
            A = [at_sb[:, jj, :] for jj in range(4)]
            v = apool.tile([128, 4], F32, tag="v")
            if c10 == 0.0 and c2s == 0.0:
                ones = apool.tile([128, 4], F32, tag="ones")
                nc.gpsimd.memset(ones[:], 1.0)
                t1 = apool.tile([128, 4], F32, tag="t1")
                nc.gpsimd.tensor_tensor(out=t1[:], in0=A[0], in1=ones[:],
                                        op=ALU.add)
                u1 = apool.tile([128, 4], F32, tag="u1")
                nc.gpsimd.tensor_tensor(out=u1[:], in0=A[1], in1=ones[:],
                                        op=ALU.add)
                t2 = apool.tile([128, 4], F32, tag="t2")
                nc.gpsimd.tensor_tensor(out=t2[:], in0=t1[:], in1=u1[:],
                                        op=ALU.mult)
                u2 = apool.tile([128, 4], F32, tag="u2")
                nc.gpsimd.tensor_tensor(out=u2[:], in0=A[2], in1=ones[:],
                                        op=ALU.add)
                t3 = apool.tile([128, 4], F32, tag="t3")
                nc.gpsimd.tensor_tensor(out=t3[:], in0=t2[:], in1=u2[:],
                                        op=ALU.mult)
                nc.gpsimd.tensor_tensor(out=v[:], in0=t3[:], in1=A[3],
                                        op=ALU.mult)
            else:
                t1 = apool.tile([128, 4], F32, tag="t1")
                nc.vector.tensor_scalar_add(t1[:], A[0], 1.0)
                t2 = apool.tile([128, 4], F32, tag="t2")
                nc.vector.scalar_tensor_tensor(
                    out=t2[:], in0=A[1], scalar=1.0, in1=t1[:],
                    op0=ALU.add, op1=ALU.mult)
                if c10 != 0.0:
                    nc.vector.tensor_scalar_add(t2[:], t2[:], c10)
                t3 = apool.tile([128, 4], F32, tag="t3")
                nc.vector.scalar_tensor_tensor(
                    out=t3[:], in0=A[2], scalar=1.0, in1=t2[:],
                    op0=ALU.add, op1=ALU.mult)
                if c2s != 0.0:
                    nc.vector.tensor_scalar_add(t3[:], t3[:], c2s)
                nc.vector.tensor_tensor(out=v[:], in0=t3[:], in1=A[3],
                                        op=ALU.mult)


            l3_pair(0, stop=False)
            l2_ph2(2)
            l2_ph2(3)
            l3_pair(1, stop=True)
            for m in range(M3):
                evict_split(h3T, l3ps[m][:], M1 + M2, m)

            # --- head: hd[b] = ow2^T h3[b] per batch tile (DR, N=1);
            # hdps recycles the matvec bank: pool WAR on the at eviction
            # orders head's bank reuse after the at values are read out ---
            hdps = psmm.tile([128, BC], F32, tag="mm", name="hdps")
            hd_ps = hdps[:, 0:4]
            for bt in range(4):
                bsl = slice(bt * 128, bt * 128 + 128)
                nc.tensor.matmul(
                    out=hd_ps[:, bt:bt + 1], lhsT=h3T[:, 0:2, bsl],
                    rhs=ws_sb[:, KD:KD + 2, 0:1], start=(bt == 0),
                    stop=(bt == 3), perf_mode=DR, skip_group_check=True)

            # --- final combine + sigmoid + out ---
            lg = apool.tile([128, 4], F32, tag="lg")
            nc.vector.scalar_tensor_tensor(
                out=lg[:], in0=hd_ps[:], scalar=INV_S2, in1=v[:],
                op0=ALU.mult, op1=ALU.add)
            if cCb != 0.0:
                nc.vector.tensor_scalar_add(lg[:], lg[:], cCb)
            res = apool.tile([128, 4], F32, tag="res")
            nc.scalar.activation(out=res[:], in_=lg[:], func=AF.Sigmoid,
                                 bias=zero_sb[:])
            nc.sync.dma_start(a["out"][:], res[:])

        if R == 1:
            _compute(_loads())
        elif BODY == "full":
            with tc.For_i(0, R, 1):
                _compute(_loads())
        elif BODY == "compute":
            T = _loads()
            with tc.For_i(0, R, 1):
                _compute(T)
        elif BODY == "dma":
            with tc.For_i(0, R, 1):
                _loads()
        else:
            raise ValueError(BODY)


def build_program(flags):
    key = ("nc", flags, os.environ.get("K_REPEAT", "1"),
           os.environ.get("K_BODY", "full"))
    if key in _CACHE:
        return _CACHE[key]
    nc = bacc.Bacc("TRN2", target_bir_lowering=False, debug=False,
                   num_devices=NC)
    aps = {}

    def din(name, shape, dt):
        aps[name] = nc.dram_tensor(name, shape, dt, kind="ExternalInput").ap()

    din("x0T", [128, KD, BC], FP8)
    din("w1x", [128, KD, M1, 128], FP8)
    din("w2x", [128, M2, K2P, 2, 128], FP8)
    din("w3x", [128, M3, K3P, 2, 128], FP8)
    din("wsx", [128, KD + 2, 4], FP8)
    if flags[3]:
        din("ballx", [128, M1 + M2 + M3], F32)
    aps["out"] = nc.dram_tensor("out", [1 * 128, 4], F32,
                                kind="ExternalOutput").ap()
    _CACHE["aps"] = aps

    with tile.TileContext(nc) as tc:
        _emit(tc, flags)
    nc.compile()
    _CACHE[key] = nc
    _CACHE["nc"] = nc  # most-recent program, for test harness introspection
    return nc


def _q(x):
    import ml_dtypes
    return (np.asarray(x, np.float32) * S).astype(ml_dtypes.float8_e4m3fn)


def prepare_in_maps(inputs):
    ids = np.asarray(inputs["ids"]).astype(np.int64)
    emb = np.asarray(inputs["emb"], dtype=np.float32)
    cross_w = np.asarray(inputs["cross_w"], dtype=np.float32)
    cross_b = np.asarray(inputs["cross_b"], dtype=np.float32)
    w1 = np.asarray(inputs["w1"], dtype=np.float32)
    w2 = np.asarray(inputs["w2"], dtype=np.float32)
    w3 = np.asarray(inputs["w3"], dtype=np.float32)
    b1 = np.asarray(inputs["b1"], dtype=np.float32)
    b2 = np.asarray(inputs["b2"], dtype=np.float32)
    b3 = np.asarray(inputs["b3"], dtype=np.float32)
    out_w = np.asarray(inputs["out_w"], dtype=np.float32)
    out_b = float(np.asarray(inputs["out_b"], dtype=np.float32))

    # cross-net constants (weight-only): CB[j,i] = W4[:,j] . cross_b[i]
    c10 = float(cross_w[1] @ cross_b[0])
    c2s = float(cross_w[2] @ (cross_b[0] + cross_b[1]))
    cCb = float(out_w[:D, 0] @ cross_b.sum(axis=0)) + out_b
    has_bias = bool(np.any(b1) or np.any(b2) or np.any(b3))
    flags = (c10, c2s, cCb, has_bias)

    x0 = emb[ids.reshape(-1)].reshape(B, D)  # [4096, 1664] f32

    w4 = np.concatenate([cross_w.T, out_w[:D].reshape(D, 1)], axis=1)
    # [K, M] -> [128, kt, M-free] k-major  (w1: [128, kt, m, 128])
    w1x = np.ascontiguousarray(
        _q(w1).reshape(KD, 128, M1, 128).transpose(1, 0, 2, 3))
    # [K, M] -> [128, m, kpair, 2, 128]
    w2x = np.ascontiguousarray(
        _q(w2).reshape(K2P, 2, 128, M2, 128).transpose(2, 3, 0, 1, 4))
    w3x = np.ascontiguousarray(
        _q(w3).reshape(K3P, 2, 128, M3, 128).transpose(2, 3, 0, 1, 4))
    import ml_dtypes
    wsx = np.zeros((128, KD + 2, 4), dtype=ml_dtypes.float8_e4m3fn)
    wsx[:, 0:KD, :] = _q(w4).reshape(KD, 128, 4).transpose(1, 0, 2)
    # ow2 as 2 k-tiles of 1 col: [p, kt] at ws[:, 13:15, 0]
    wsx[:, KD:KD + 2, 0] = _q(out_w[D:]).reshape(M3, 128).T
    shared = dict(w1x=w1x, w2x=w2x, w3x=w3x, wsx=np.ascontiguousarray(wsx))
    if has_bias:
        ballx = np.zeros((128, M1 + M2 + M3), dtype=np.float32)
        ballx[:, 0:M1] = S * b1.reshape(M1, 128).T
        ballx[:, M1:M1 + M2] = S * b2.reshape(M2, 128).T
        ballx[:, M1 + M2:] = S * b3.reshape(M3, 128).T
        shared["ballx"] = np.ascontiguousarray(ballx)

    in_maps = []
    for c in range(NC):
        xc = _q(x0[c * BC:(c + 1) * BC])                  # [512, 1664] fp8
        x0T = np.ascontiguousarray(
            xc.reshape(BC, KD, 128).transpose(2, 1, 0))   # [128, 13, 512]
        in_maps.append(dict(x0T=x0T, **shared))
    return in_maps, flags


def kernel(**inputs):
    in_maps, flags = prepare_in_maps(inputs)
    nc = build_program(flags)
    res = bass_utils.run_bass_kernel_spmd(nc, in_maps, core_ids=list(range(NC)))
    out = np.empty((NC, BC), dtype=np.float32)
    for c in range(NC):
        o = res.results[c]["out"]            # [128, 4] -> example bt*128+p
        out[c] = o.T.reshape(BC)
    return out.reshape(B, 1).astype(np.float32)


# revision 35
# speedup vs baseline: 1.0517x; 1.0310x over previous
"""Deep & Cross Network kernel for 8x Trainium2 NeuronCores (Bass/Tile).

Sharding: pure data-parallel over batch (512 rows/core); weights replicated
per core; no collectives (cost model charges >=15us constant per collective).

Host-side prep inside kernel() (layout/dtype only):
  - embedding lookup x0 = emb[ids] (pure gather), transpose per core to
    x0T [128, 13, 512] and quantize to fp8e4m3 at scale S=32
  - weights pre-tiled to SBUF-native lhsT layouts, fp8 at scale S
  - cross-net bias constants c10/c2s/cC collapse to three scalars (host dot
    products of weight-only data); they are 0 for the reference's zero
    cross_b and fold into immediates

Device math (per core, n=512, D=1664, S=32):
  All matmuls fp8 DoubleRow (2 k-tiles per instruction, 0.5 cyc/row).
  Deep tower weights-stationary: psum = (S x0)(S w) = S^2 * pre;
  evict h = relu(psum)/S stored fp8 (scale S) feeds the next layer.
  Cross net in batch-on-partition layout [128b, 4bt]:
    A = (x0q @ [w0 w1 w2 ow[:D]]) / S^2  -> [128, 4] per batch-tile
    t1 = 1+a0; t2 = t1*(1+a1)+c10; t3 = t2*(1+a2)+c2s
    logit = t3*a3 + cC+out_b + (h3 @ ow[D:]) ; out = sigmoid(logit)
  PE warmup matmuls during the DMA head burn the p-state ramp.
"""

import os
import sys
import numpy as np

for _p in ("/opt/trn_rl_repo",):
    if _p not in sys.path:
        sys.path.insert(0, _p)

import concourse.bass as bass
import concourse.tile as tile
from concourse import bacc, mybir
from concourse import bass_utils

F32 = mybir.dt.float32
FP8 = mybir.dt.float8e4
AF = mybir.ActivationFunctionType
ALU = mybir.AluOpType
DR = mybir.MatmulPerfMode.DoubleRow

B, F, E, H = 4096, 26, 64, 1000000
D = F * E            # 1664
NC = 8
BC = B // NC         # 512 rows per core
KD = D // 128        # 13 k-tiles over D
H1, H2, H3 = 1024, 512, 256
M1, M2, M3 = H1 // 128, H2 // 128, H3 // 128
K2P, K3P = H1 // 256, H2 // 256   # DR k-pairs for L2/L3
NP1 = (KD - 1) // 2               # 6 DR pairs for L1 (kt0 single first)
S = 32.0
INV_S = 1.0 / S
INV_S2 = 1.0 / (S * S)
NWARM = int(os.environ.get("K_WARM", "28"))

_CACHE = {}


def _emit(tc, flags):
    nc = tc.nc
    a = _CACHE["aps"]
    c10, c2s, cCb, has_bias = flags
    R = int(os.environ.get("K_REPEAT", "1"))
    BODY = os.environ.get("K_BODY", "full")  # full | compute | dma

    with (
        tc.tile_pool(name="const", bufs=1) as cpool,
        tc.tile_pool(name="act", bufs=1) as apool,
        tc.tile_pool(name="psmm", bufs=7, space="PSUM") as psmm,
        tc.tile_pool(name="pssm", bufs=1, space="PSUM") as pssm,
    ):
        def _loads():
            x0_sb = cpool.tile([128, KD, BC], FP8, tag="x0")
            w1_sb = cpool.tile([128, KD, M1, 128], FP8, tag="w1")
            # kt-chunked x0/w1 interleave; chunk sizes chosen so transfers
            # stream without HWDGE slot gaps (~650ns/DMA issue cadence)
            for lo, hi in ((0, 1), (1, 4), (4, 7), (7, 10), (10, 13)):
                nc.sync.dma_start(x0_sb[:, lo:hi, :], a["x0T"][:, lo:hi, :])
                nc.sync.dma_start(w1_sb[:, lo:hi, :, :], a["w1x"][:, lo:hi, :, :])
            # w2 kt-major, split so L2 phase 1 weights land first
            w2_sb = cpool.tile([128, K2P, 2, M2, 128], FP8, tag="w2")
            nc.sync.dma_start(w2_sb[:, 0:2, :, :, :], a["w2x"][:, 0:2, :, :, :])
            nc.sync.dma_start(w2_sb[:, 2:4, :, :, :], a["w2x"][:, 2:4, :, :, :])
            w3_sb = cpool.tile([128, M3, K3P, 2, 128], FP8, tag="w3")
            nc.sync.dma_start(w3_sb[:], a["w3x"][:])
            # ws: w4 in [:, 0:13, :], ow2 in [:, 13:15, 0] - needed latest
            ws_sb = cpool.tile([128, KD + 2, 4], FP8, tag="ws")
            nc.sync.dma_start(ws_sb[:], a["wsx"][:])
            T = dict(ws_sb=ws_sb, x0_sb=x0_sb, w1_sb=w1_sb,
                     w2_sb=w2_sb, w3_sb=w3_sb)
            if has_bias:
                ball_sb = cpool.tile([128, M1 + M2 + M3], F32, tag="ball")
                nc.sync.dma_start(ball_sb[:], a["ballx"][:])
                T["ball_sb"] = ball_sb
            return T

        def _compute(T):
            ws_sb = T["ws_sb"]; x0_sb = T["x0_sb"]; w1_sb = T["w1_sb"]
            w2_sb = T["w2_sb"]; w3_sb = T["w3_sb"]
            ball = T.get("ball_sb")

            def bias_ap(layer_off, m):
                return ball[:, layer_off + m:layer_off + m + 1]

            # --- engine warmups (all independent of loads) ---
            warm = apool.tile([128, 128], FP8, tag="warm")
            nc.gpsimd.memset(warm[:], 0.0)
            zero_sb = apool.tile([128, 1], F32, tag="zero")
            nc.vector.memset(zero_sb[:], 0.0)
            # ACT function tables (Sigmoid set also contains Relu/Copy)
            wa = apool.tile([128, 1], F32, tag="wa")
            nc.scalar.activation(out=wa[:], in_=zero_sb[:], func=AF.Sigmoid,
                                 bias=zero_sb[:])
            wr = apool.tile([128, 1], F32, tag="wr")
            nc.scalar.activation(out=wr[:], in_=zero_sb[:], func=AF.Relu,
                                 bias=zero_sb[:])
            # single PSUM bank shared by warmup/matvec/head outputs. A
            # start=True zero-marks the whole bank here, so each region gets
            # exactly one group start per live window; the warmup sweep also
            # writes zeros over the full bank so start=False accumulation
            # into fresh regions is safe under element-wise-zero semantics.
            arena = pssm.tile([128, 512], F32, tag="sm", name="arena")
            # PE p-state ramp burn: back-to-back junk matmuls sweeping arena
            for i in range(max(NWARM, 4)):
                c = (i % 4) * 128
                nc.tensor.matmul(out=arena[:, c:c + 128], lhsT=warm[:],
                                 rhs=warm[:], start=True, stop=True,
                                 skip_group_check=True)

            # --- band helpers: band 0 = kt0 single, band j = DR pair ---
            def mm_band(j, ps, lhsT_kt, rhs_kt, stop, start=None, skip=False):
                """lhsT_kt/rhs_kt: callables band-slice -> AP"""
                if start is None:
                    start = j == 0
                if j == 0:
                    nc.tensor.matmul(out=ps, lhsT=lhsT_kt(0, 1),
                                     rhs=rhs_kt(0, 1), start=start, stop=stop,
                                     skip_group_check=skip)
                else:
                    lo, hi = 2 * j - 1, 2 * j + 1
                    nc.tensor.matmul(out=ps, lhsT=lhsT_kt(lo, hi),
                                     rhs=rhs_kt(lo, hi), start=start,
                                     stop=stop, perf_mode=DR,
                                     skip_group_check=skip)

            def x0sl(lo, hi):
                return x0_sb[:, lo:hi, :] if hi - lo == 2 else x0_sb[:, lo, :]

            # --- eviction helpers ---
            h1T = apool.tile([128, M1, BC], FP8, tag="h1T")
            h2T = apool.tile([128, M2, BC], FP8, tag="h2T")
            h3T = apool.tile([128, M3, BC], FP8, tag="h3T")

            def evict(dst, ps, eng, loff, m):
                """dst = relu(ps * 1/S [+ S*b]) on the given engine."""
                if eng == "act":
                    nc.scalar.activation(
                        out=dst, in_=ps, func=AF.Relu, scale=INV_S,
                        bias=bias_ap(loff, m) if has_bias else zero_sb[:])
                elif not has_bias:
                    nc.vector.tensor_scalar(
                        out=dst, in0=ps, scalar1=INV_S,
                        scalar2=0.0, op0=ALU.mult, op1=ALU.max)
                else:
                    nc.vector.tensor_scalar(
                        out=dst, in0=ps, scalar1=INV_S,
                        scalar2=bias_ap(loff, m), op0=ALU.mult, op1=ALU.add)
                    nc.vector.tensor_scalar_max(out=dst, in0=dst, scalar1=0.0)

            def evict_split(dst3, ps, loff, m):
                """latency-critical eviction: halves on ACT + DVE."""
                h = BC // 2
                evict(dst3[:, m, 0:h], ps[:, 0:h], "act", loff, m)
                evict(dst3[:, m, h:BC], ps[:, h:BC], "dve", loff, m)

            # --- L1: 8 PSUM banks (7 pool + arena), band-major bands 0-5,
            # band 6 m-major with immediate staggered evictions ---
            l1ps = [psmm.tile([128, BC], F32, tag="mm", name=f"l1_{m}")
                    for m in range(7)] + [arena]

            def w1lh(lo, hi, m):
                return (w1_sb[:, lo:hi, m, :] if hi - lo == 2
                        else w1_sb[:, lo, m, :])

            for j in range(NP1 - 2):
                for m in range(8):
                    mm_band(j, l1ps[m][:], lambda lo, hi, m=m: w1lh(lo, hi, m),
                            x0sl, stop=False)
            # last three bands m-major so evictions stagger into the L1 tail
            for m in range(8):
                for j in range(NP1 - 2, NP1 + 1):
                    mm_band(j, l1ps[m][:],
                            lambda lo, hi, m=m: w1lh(lo, hi, m), x0sl,
                            stop=(j == NP1))
                evict(h1T[:, m, :], l1ps[m][:], ("act", "dve")[m % 2], 0, m)

            # --- cross matvec (batch-on-partition), SBUF-fed post-L1;
            # accumulates in a recycled bank (one bank-wide group) ---
            atps = psmm.tile([128, BC], F32, tag="mm", name="atps")
            at_ps = atps[:, 0:16]
            for j in range(NP1 + 1):
                for bt in range(4):
                    bsl = slice(bt * 128, bt * 128 + 128)

                    def xlh(lo, hi, bsl=bsl):
                        return (x0_sb[:, lo:hi, bsl] if hi - lo == 2
                                else x0_sb[:, lo, bsl])

                    def wrh(lo, hi):
                        return (ws_sb[:, lo:hi, :] if hi - lo == 2
                                else ws_sb[:, lo, :])

                    mm_band(j, at_ps[:, 4 * bt:4 * bt + 4], xlh, wrh,
                            stop=(j == NP1 and bt == 3),
                            start=(j == 0 and bt == 0), skip=True)

            # --- L2 in two k-phases; phase 2 staggers h2 evictions and
            # interleaves L3 pairs as their h2 inputs land ---
            l2ps = [psmm.tile([128, BC], F32, tag="mm", name=f"l2_{m}")
                    for m in range(M2)]
            for m in range(M2):
                for t in range(2):
                    nc.tensor.matmul(
                        out=l2ps[m][:], lhsT=w2_sb[:, t, :, m, :],
                        rhs=h1T[:, 2 * t:2 * t + 2, :],
                        start=(t == 0), stop=False, perf_mode=DR)
            l3ps = [psmm.tile([128, BC], F32, tag="mm", name=f"l3_{m}")
                    for m in range(M3)]

            def l2_ph2(m):
                for t in range(2, K2P):
                    nc.tensor.matmul(
                        out=l2ps[m][:], lhsT=w2_sb[:, t, :, m, :],
                        rhs=h1T[:, 2 * t:2 * t + 2, :],
                        start=False, stop=(t == K2P - 1), perf_mode=DR)
                evict_split(h2T, l2ps[m][:], M1, m)

            def l3_pair(t, stop):
                for m in range(M3):
                    nc.tensor.matmul(
                        out=l3ps[m][:], lhsT=w3_sb[:, m, t, :, :],
                        rhs=h2T[:, 2 * t:2 * t + 2, :],
                        start=(t == 0), stop=stop, perf_mode=DR)

            l2_ph2(0)
            l2_ph2(1)
            # at eviction (DVE, reads PSUM) + cross recurrence. Fast path
            # runs the recurrence on the otherwise-idle Pool engine with
            # tensor_tensor ops only (Pool ISA has no tensor_scalar);
            # general path (nonzero cross_b consts) falls back to DVE.
            # at_sb layout [128, j, bt] so A_j slices are contiguous
            # (GPSIMD chokes on strided reads); DVE does the strided write
            at_sb = apool.tile([128, 4, 4], F32, tag="at")
            for bt in range(4):
                nc.vector.tensor_scalar_mul(
                    at_sb[:, :, bt], at_ps[:, 4 * bt:4 * bt + 4], INV_S2)
            A = [at_sb[:, jj, :] for jj in range(4)]
            v = apool.tile([128, 4], F32, tag="v")
            if c10 == 0.0 and c2s == 0.0:
                ones = apool.tile([128, 4], F32, tag="ones")
                nc.gpsimd.memset(ones[:], 1.0)
                t1 = apool.tile([128, 4], F32, tag="t1")
                nc.gpsimd.tensor_tensor(out=t1[:], in0=A[0], in1=ones[:],
                                        op=ALU.add)
                u1 = apool.tile([128, 4], F32, tag="u1")
                nc.gpsimd.tensor_tensor(out=u1[:], in0=A[1], in1=ones[:],
                                        op=ALU.add)
                t2 = apool.tile([128, 4], F32, tag="t2")
                nc.gpsimd.tensor_tensor(out=t2[:], in0=t1[:], in1=u1[:],
                                        op=ALU.mult)
                u2 = apool.tile([128, 4], F32, tag="u2")
                nc.gpsimd.tensor_tensor(out=u2[:], in0=A[2], in1=ones[:],
                                        op=ALU.add)
                t3 = apool.tile([128, 4], F32, tag="t3")
                nc.gpsimd.tensor_tensor(out=t3[:], in0=t2[:], in1=u2[:],
                                        op=ALU.mult)
                nc.gpsimd.tensor_tensor(out=v[:], in0=t3[:], in1=A[3],
                                        op=ALU.mult)
            else:
                t1 = apool.tile([128, 4], F32, tag="t1")
                nc.vector.tensor_scalar_add(t1[:], A[0], 1.0)
                t2 = apool.tile([128, 4], F32, tag="t2")
                nc.vector.scalar_tensor_tensor(
                    out=t2[:], in0=A[1], scalar=1.0, in1=t1[:],
                    op0=ALU.add, op1=ALU.mult)
                if c10 != 0.0:
                    nc.vector.tensor_scalar_add(t2[:], t2[:], c10)
                t3 = apool.tile([128, 4], F32, tag="t3")
                nc.vector.scalar_tensor_tensor(
                    out=t3[:], in0=A[2], scalar=1.0, in1=t2[:],
                    op0=ALU.add, op1=ALU.mult)
                if c2s != 0.0:
                    nc.vector.tensor_scalar_add(t3[:], t3[:], c2s)
                nc.vector.tensor_tensor(out=v[:], in0=t3[:], in1=A[3],
                                        op=ALU.mult)


            l3_pair(0, stop=False)
            l2_ph2(2)
            l2_ph2(3)
            l3_pair(1, stop=True)
            for m in range(M3):
                evict_split(h3T, l3ps[m][:], M1 + M2, m)

            # --- head: hd[b] = ow2^T h3[b] per batch tile (DR, N=1);
            # hdps recycles the matvec bank: pool WAR on the at eviction
            # orders head's bank reuse after the at values are read out ---
            hdps = psmm.tile([128, BC], F32, tag="mm", name="hdps")
            hd_ps = hdps[:, 0:4]
            for bt in range(4):
                bsl = slice(bt * 128, bt * 128 + 128)
                nc.tensor.matmul(
                    out=hd_ps[:, bt:bt + 1], lhsT=h3T[:, 0:2, bsl],
                    rhs=ws_sb[:, KD:KD + 2, 0:1], start=(bt == 0),
                    stop=(bt == 3), perf_mode=DR, skip_group_check=True)

            # --- final combine + sigmoid + out ---
            lg = apool.tile([128, 4], F32, tag="lg")
            nc.vector.scalar_tensor_tensor(
                out=lg[:], in0=hd_ps[:], scalar=INV_S2, in1=v[:],
                op0=ALU.mult, op1=ALU.add)
            if cCb != 0.0:
                nc.vector.tensor_scalar_add(lg[:], lg[:], cCb)
            res = apool.tile([128, 4], F32, tag="res")
            nc.scalar.activation(out=res[:], in_=lg[:], func=AF.Sigmoid,
                                 bias=zero_sb[:])
            nc.sync.dma_start(a["out"][:], res[:])

        if R == 1:
            _compute(_loads())
        elif BODY == "full":
            with tc.For_i(0, R, 1):
                _compute(_loads())
        elif BODY == "compute":
            T = _loads()
            with tc.For_i(0, R, 1):
                _compute(T)
        elif BODY == "dma":
            with tc.For_i(0, R, 1):
                _loads()
        else:
            raise ValueError(BODY)


def build_program(flags):
    key = ("nc", flags, os.environ.get("K_REPEAT", "1"),
           os.environ.get("K_BODY", "full"))
    if key in _CACHE:
        return _CACHE[key]
    nc = bacc.Bacc("TRN2", target_bir_lowering=False, debug=False,
                   num_devices=NC)
    aps = {}

    def din(name, shape, dt):
        aps[name] = nc.dram_tensor(name, shape, dt, kind="ExternalInput").ap()

    din("x0T", [128, KD, BC], FP8)
    din("w1x", [128, KD, M1, 128], FP8)
    din("w2x", [128, K2P, 2, M2, 128], FP8)
    din("w3x", [128, M3, K3P, 2, 128], FP8)
    din("wsx", [128, KD + 2, 4], FP8)
    if flags[3]:
        din("ballx", [128, M1 + M2 + M3], F32)
    aps["out"] = nc.dram_tensor("out", [1 * 128, 4], F32,
                                kind="ExternalOutput").ap()
    _CACHE["aps"] = aps

    with tile.TileContext(nc) as tc:
        _emit(tc, flags)
    nc.compile()
    _CACHE[key] = nc
    _CACHE["nc"] = nc  # most-recent program, for test harness introspection
    return nc


def _q(x):
    import ml_dtypes
    return (np.asarray(x, np.float32) * S).astype(ml_dtypes.float8_e4m3fn)


def prepare_in_maps(inputs):
    ids = np.asarray(inputs["ids"]).astype(np.int64)
    emb = np.asarray(inputs["emb"], dtype=np.float32)
    cross_w = np.asarray(inputs["cross_w"], dtype=np.float32)
    cross_b = np.asarray(inputs["cross_b"], dtype=np.float32)
    w1 = np.asarray(inputs["w1"], dtype=np.float32)
    w2 = np.asarray(inputs["w2"], dtype=np.float32)
    w3 = np.asarray(inputs["w3"], dtype=np.float32)
    b1 = np.asarray(inputs["b1"], dtype=np.float32)
    b2 = np.asarray(inputs["b2"], dtype=np.float32)
    b3 = np.asarray(inputs["b3"], dtype=np.float32)
    out_w = np.asarray(inputs["out_w"], dtype=np.float32)
    out_b = float(np.asarray(inputs["out_b"], dtype=np.float32))

    # cross-net constants (weight-only): CB[j,i] = W4[:,j] . cross_b[i]
    c10 = float(cross_w[1] @ cross_b[0])
    c2s = float(cross_w[2] @ (cross_b[0] + cross_b[1]))
    cCb = float(out_w[:D, 0] @ cross_b.sum(axis=0)) + out_b
    has_bias = bool(np.any(b1) or np.any(b2) or np.any(b3))
    flags = (c10, c2s, cCb, has_bias)

    x0 = emb[ids.reshape(-1)].reshape(B, D)  # [4096, 1664] f32

    w4 = np.concatenate([cross_w.T, out_w[:D].reshape(D, 1)], axis=1)
    # [K, M] -> [128, kt, M-free] k-major  (w1: [128, kt, m, 128])
    w1x = np.ascontiguousarray(
        _q(w1).reshape(KD, 128, M1, 128).transpose(1, 0, 2, 3))
    # [K, M] -> [128, m, kpair, 2, 128]
    w2x = np.ascontiguousarray(
        _q(w2).reshape(K2P, 2, 128, M2, 128).transpose(2, 0, 1, 3, 4))
    w3x = np.ascontiguousarray(
        _q(w3).reshape(K3P, 2, 128, M3, 128).transpose(2, 3, 0, 1, 4))
    import ml_dtypes
    wsx = np.zeros((128, KD + 2, 4), dtype=ml_dtypes.float8_e4m3fn)
    wsx[:, 0:KD, :] = _q(w4).reshape(KD, 128, 4).transpose(1, 0, 2)
    # ow2 as 2 k-tiles of 1 col: [p, kt] at ws[:, 13:15, 0]
    wsx[:, KD:KD + 2, 0] = _q(out_w[D:]).reshape(M3, 128).T
    shared = dict(w1x=w1x, w2x=w2x, w3x=w3x, wsx=np.ascontiguousarray(wsx))
    if has_bias:
        ballx = np.zeros((128, M1 + M2 + M3), dtype=np.float32)
        ballx[:, 0:M1] = S * b1.reshape(M1, 128).T
        ballx[:, M1:M1 + M2] = S * b2.reshape(M2, 128).T
        ballx[:, M1 + M2:] = S * b3.reshape(M3, 128).T
        shared["ballx"] = np.ascontiguousarray(ballx)

    in_maps = []
    for c in range(NC):
        xc = _q(x0[c * BC:(c + 1) * BC])                  # [512, 1664] fp8
        x0T = np.ascontiguousarray(
            xc.reshape(BC, KD, 128).transpose(2, 1, 0))   # [128, 13, 512]
        in_maps.append(dict(x0T=x0T, **shared))
    return in_maps, flags


def kernel(**inputs):
    in_maps, flags = prepare_in_maps(inputs)
    nc = build_program(flags)
    res = bass_utils.run_bass_kernel_spmd(nc, in_maps, core_ids=list(range(NC)))
    out = np.empty((NC, BC), dtype=np.float32)
    for c in range(NC):
        o = res.results[c]["out"]            # [128, 4] -> example bt*128+p
        out[c] = o.T.reshape(BC)
    return out.reshape(B, 1).astype(np.float32)
